# revision 55
# baseline (speedup 1.0000x reference)
"""Trainium2 Bass kernel for nn_MultiHeadCausalAttention (B=4, S=2048, D=1024, H=16).

Sharding: 8 cores = 4 (batch) x 2 (tensor-parallel over heads; 8 heads/core).
Per core (vs. the earlier AllGather design):
  - x arrives untransposed; x^T tiles are built on-chip with the DMA
    transpose XBAR (no host-side transpose).
  - QKV projections for the core's 8 heads.
  - Flash-style causal attention, scores in S^T layout (k on partitions) so
    exp(S^T) tiles feed the AV matmuls as lhsT.  AV outputs A in [q, d]
    layout (queries on partitions) via per-128-query-tile matmuls whose rhs
    is V|ones, so softmax row sums ride along in an extra column.
  - Normalisation is a reciprocal + broadcast multiply on DVE writing
    normalised A (bf16) to SBUF; the DMA XBAR transposes A -> A^T tiles.
  - Partial out-projection over the LOCAL 512 head-dims for all 1024 output
    columns, then a ReduceScatter(add) per 512-query chunk combines the two
    TP halves — the collective is the last step of each chunk and overlaps
    the attention of later chunks (query-block-major schedule).
  - Out-projection matmuls are interleaved one-per-attention-step into the
    next query block so the PE never idles waiting for exp.
Raw Bass (no Tile): per-engine programs with hand-placed counting semaphores.
`reps` replays the body inside one NEFF (sem values offset per rep) so the
true per-iteration time can be measured as a slope, independent of the
axon dispatch floor.
"""

from contextlib import ExitStack

import numpy as np
import ml_dtypes

import concourse.bass as bass
import concourse.mybir as mybir
from concourse.bass_utils import run_bass_kernel_spmd

F32 = mybir.dt.float32
BF16 = mybir.dt.bfloat16
AF = mybir.ActivationFunctionType

B, S_FULL, D = 4, 2048, 1024
NCORES = 8
NDT = D // 128         # 8 x-tiles along the model dim
DOWN = D // 2          # head-dims owned per core (8 heads * 64)
SCALE = 1.0 / 32.0     # d_out ** -0.5
RG = [[0, 1], [2, 3], [4, 5], [6, 7]]
NHP = 4                # head pairs per core


class Waiter:
    """Per-engine wait helper that elides waits already implied."""

    def __init__(self, eng):
        self.eng = eng
        self.seen = {}

    def __call__(self, sem, val):
        if val <= 0:
            return
        if self.seen.get(sem.name, -1) >= val:
            return
        self.seen[sem.name] = val
        self.eng.wait_ge(sem, val)


def apv(base_ap, dims):
    """Manual AP: keep base's partition dim, replace free dims ([stride, count])."""
    return bass.AP(base_ap.tensor, base_ap.offset, [list(base_ap.ap[0])] + dims)


def build_program(S=S_FULL, reps=1, debug=False, no_coll=False):
    NQB = S // 512         # query blocks
    NST = S // 128         # seq tiles
    NBLK = NQB * NHP       # (qb, hp) blocks per rep

    # qb-major attention step list
    steps = []
    block_last = []        # last step index of each block
    for qb in range(NQB):
        nkt = (qb + 1) * 4
        for hp in range(NHP):
            for kt in range(nkt):
                delta = kt * 128 - qb * 512
                qt_min = max(0, (delta + 127) // 128)
                steps.append((qb, hp, kt, nkt, delta, qt_min))
            block_last.append(len(steps) - 1)
    NSTEPS = len(steps)

    # last attention step reading QT/KT of hp (scores) and VS[st] (AV)
    hp_last = [max(i for i, s in enumerate(steps) if s[1] == hp) for hp in range(NHP)]
    vs_last = [max(i for i, s in enumerate(steps) if s[2] == st) for st in range(NST)]

    # proj emission order shared by PE and DVE
    proj = []
    for hp in range(NHP):
        for sb in range(S // 512):
            proj.append(("q", hp, sb))
            proj.append(("k", hp, sb))
    for st in range(NST):
        proj.append(("v", st))
    NPJ = len(proj)

    # psum column map (128 x 4096 f32 = 8 banks of 512)
    AVB = 2048             # AV regions in banks 4-5

    def avcol(qt):
        return AVB + (qt // 2) * 512 + (qt % 2) * 130

    OPB = 3072             # out-proj regions ping-pong banks 6-7
    RSU = 16 if no_coll else 1

    nc = bass.Bass()
    xb = nc.declare_dram_parameter("xb", [S, D], BF16, isOutput=False)
    wq = nc.declare_dram_parameter("wq", [D, DOWN], BF16, isOutput=False)
    wk = nc.declare_dram_parameter("wk", [D, DOWN], BF16, isOutput=False)
    wv = nc.declare_dram_parameter("wv", [D, DOWN], BF16, isOutput=False)
    wo2 = nc.declare_dram_parameter("wo2", [DOWN, D], BF16, isOutput=False)
    bob2 = nc.declare_dram_parameter("bob2", [128, 4 * DOWN], F32, isOutput=False)
    ltri = nc.declare_dram_parameter("ltri", [128, 128], BF16, isOutput=False)
    negi = nc.declare_dram_parameter("negi", [128, 128], BF16, isOutput=False)
    out = nc.declare_dram_parameter("out", [S, DOWN], F32, isOutput=True)

    # cci rows: qb*1024 + half*512 + local_q — each qb's ReduceScatter input
    # is the contiguous [1024, 512] block (dest-rank halves adjacent)
    cci = nc.dram_tensor("cci_rs", [2 * S, DOWN], BF16)
    cco = nc.dram_tensor("cco_rs", [S, DOWN], BF16)
    dbg = (
        nc.declare_dram_parameter("dbg", [2 * S, DOWN], BF16, isOutput=True)
        if debug
        else None
    )
    dbg2 = (
        nc.declare_dram_parameter("dbg2", [128, 2048], BF16, isOutput=True)
        if debug
        else None
    )
    dbg3 = (
        nc.declare_dram_parameter("dbg3", [128, 2048], BF16, isOutput=True)
        if debug
        else None
    )
    dbg4 = (
        nc.declare_dram_parameter("dbg4", [128, 1024], F32, isOutput=True)
        if debug
        else None
    )

    with ExitStack() as ctx:
        e = ctx.enter_context
        ctx.enter_context(
            nc.allow_low_precision(reason="intentional bf16 flash attention")
        )

        sems = {}
        for n in (
            "dXQ", "dWK", "dWV", "dWO", "dMISC", "sPJ", "sPJC", "sPS", "sEX",
            "sAV", "sNM", "dAT", "sOP", "sOD", "dOC", "sRS", "dCI", "sOB", "dO",
            "sDB",
        ):
            sems[n] = e(nc.semaphore(n))
        sDB = sems["sDB"]
        dXQ, dWK, dWV, dWO, dMISC = (sems[k] for k in ("dXQ", "dWK", "dWV", "dWO", "dMISC"))
        sPJ, sPJC, sPS, sEX, sAV = (sems[k] for k in ("sPJ", "sPJC", "sPS", "sEX", "sAV"))
        sNM, dAT, sOP, sOD, dOC = (sems[k] for k in ("sNM", "dAT", "sOP", "sOD", "dOC"))
        sRS, dCI, sOB, dO = (sems[k] for k in ("sRS", "dCI", "sOB", "dO"))

        P = e(nc.psum_tensor("P", [128, 4096], F32))

        XT = [e(nc.sbuf_tensor(f"XT{i}", [128, S], BF16)) for i in range(NDT)]
        WQt = [e(nc.sbuf_tensor(f"WQt{i}", [128, DOWN], BF16)) for i in range(NDT)]
        WKt = [e(nc.sbuf_tensor(f"WKt{i}", [128, DOWN], BF16)) for i in range(NDT)]
        WVt = [e(nc.sbuf_tensor(f"WVt{i}", [128, DOWN], BF16)) for i in range(NDT)]
        WO2 = [e(nc.sbuf_tensor(f"WO2_{i}", [128, D], BF16)) for i in range(4)]
        QT = [e(nc.sbuf_tensor(f"QT{i}", [128, S], BF16)) for i in range(NHP)]
        KT = [e(nc.sbuf_tensor(f"KT{i}", [128, S], BF16)) for i in range(NHP)]
        VS = [e(nc.sbuf_tensor(f"VS{i}", [128, 8 * 65], BF16)) for i in range(NST)]
        PT = [e(nc.sbuf_tensor(f"PT{i}", [128, 1024], BF16)) for i in range(6)]
        RCM = e(nc.sbuf_tensor("RCM", [128, 16], F32))
        ASB = e(nc.sbuf_tensor("ASB", [128, 2048], BF16))
        ATB1 = e(nc.sbuf_tensor("ATB1", [128, 2048], BF16))
        OSB = [e(nc.sbuf_tensor(f"OSB{i}", [128, 512], BF16)) for i in range(4)]
        CSB = [e(nc.sbuf_tensor(f"CSB{i}", [128, 2048], BF16)) for i in range(2)]
        OUB = [e(nc.sbuf_tensor(f"OUB{i}", [128, 2048], F32)) for i in range(2)]
        DBGP = e(nc.sbuf_tensor("DBGP", [128, 1024], F32)) if debug else None
        LTRI = e(nc.sbuf_tensor("LTRI", [128, 128], BF16))
        NEGI = e(nc.sbuf_tensor("NEGI", [128, 128], BF16))
        BOB = e(nc.sbuf_tensor("BOB", [128, 4 * DOWN], F32))

        with nc.Block() as blk:

            @blk.sync
            def _(sync):
                w = Waiter(sync)
                for r in range(reps):
                    if r == 0:
                        for i in range(NDT):
                            sync.dma_start_transpose(
                                XT[i][:], xb[:, i * 128 : (i + 1) * 128]
                            ).then_inc(dXQ, 16)
                        for i in range(NDT):
                            sl = slice(i * 128, (i + 1) * 128)
                            sync.dma_start(WQt[i][:], wq[sl, :]).then_inc(dXQ, 16)
                        for i in range(NDT):
                            sl = slice(i * 128, (i + 1) * 128)
                            sync.dma_start(WKt[i][:], wk[sl, :]).then_inc(dWK, 16)
                        for i in range(NDT):
                            sl = slice(i * 128, (i + 1) * 128)
                            sync.dma_start(WVt[i][:], wv[sl, :]).then_inc(dWV, 16)
                        for i in range(4):
                            sl = slice(i * 128, (i + 1) * 128)
                            sync.dma_start(WO2[i][:], wo2[sl, :]).then_inc(dWO, 16)
                        sync.dma_start(LTRI[:], ltri[:]).then_inc(dMISC, 16)
                        sync.dma_start(NEGI[:], negi[:]).then_inc(dMISC, 16)
                        sync.dma_start(BOB[:], bob2[:]).then_inc(dMISC, 16)
                    # A^T tile transposes per query block: ASB [q, (qt,dt,d)]
                    # -> ATB1 [d, (dt,qt,q)] via the DMA XBAR, one call per qt
                    for qb in range(NQB):
                        if dbg2 is not None and r == 0 and qb == 0:
                            w(sNM, 2 * (r * NBLK + (qb + 1) * NHP))
                            sync.dma_start(dbg2[:], ASB[:]).then_inc(dMISC, 16)
                        if dbg3 is not None and r == 0 and qb == 1:
                            w(sNM, 2 * (r * NBLK + (qb + 1) * NHP))
                            sync.dma_start(dbg3[:], ATB1[:]).then_inc(dMISC, 16)
                            w(dMISC, 80)  # dbg3 read before overwrite
                        for qt in range(4):
                            w(sNM, 2 * (r * NBLK + (qb + 1) * NHP))
                            if qt == 0:
                                w(sOP, 8 * (r * NQB + qb))  # ATB1 free
                            sync.dma_start_transpose(
                                apv(ATB1[:, qt * 128 : qt * 128 + 1], [[512, 4], [1, 128]]),
                                ASB[:, qt * 512 : (qt + 1) * 512],
                            ).then_inc(dAT, 16)
                    if r + 1 < reps:
                        w(sPJ, (r + 1) * NPJ)  # this rep's proj done reading XT
                        for i in range(NDT):
                            sync.dma_start_transpose(
                                XT[i][:], xb[:, i * 128 : (i + 1) * 128]
                            ).then_inc(dXQ, 16)
                    for qb in range(NQB):
                        gq = r * NQB + qb
                        w(sOB, gq + 1)
                        ov = out[qb * 512 : (qb + 1) * 512, :].rearrange(
                            "(a b) c -> b a c", a=4
                        )
                        uv = OUB[qb % 2][:, :].rearrange("p (a c) -> p a c", a=4)
                        sync.dma_start(ov, uv).then_inc(dO, 16)
                w(dO, 16 * reps * NQB)
                if dbg is not None:
                    w(dOC, 16 * reps * 32)
                    sync.dma_start(dbg[:], cci[:]).then_inc(dMISC, 16)
                if dbg4 is not None:
                    w(sDB, 1)
                    sync.dma_start(dbg4[:], DBGP[:]).then_inc(dMISC, 16)

            @blk.gpsimd
            def _(gpsimd):
                w = Waiter(gpsimd)

                def bias_add(r, qb):
                    # cco+bias -> OUB on Pool (SBUF-only engine; keeps the
                    # rep-tail off DVE so the next rep's proj copies can run)
                    gq = r * NQB + qb
                    w(dCI, 16 * (gq + 1))
                    w(dO, 16 * (gq - 1))  # OUB[qb%2] free
                    gpsimd.tensor_add(
                        OUB[qb % 2][:], CSB[qb % 2][:], BOB[:]
                    ).then_inc(sOB, 1)

                for r in range(reps):
                    for qb in range(NQB):
                        if qb == NQB - 1 and r * NQB + qb >= 3:
                            bias_add(*divmod(r * NQB + qb - 3, NQB))
                        for g in range(8):
                            qt, half = g // 2, g % 2
                            gidx = r * 32 + qb * 8 + g
                            w(sOD, gidx + 1)
                            if g == 0:
                                w(sRS, RSU * ((r - 1) * NQB + qb + 1))  # cci chunk free
                            row = qb * 1024 + half * 512 + qt * 128
                            gpsimd.dma_start(
                                cci[row : row + 128, :],
                                OSB[gidx % 4][:],
                            ).then_inc(dOC, 16)
                        w(dOC, 16 * (r * 32 + qb * 8 + 8))
                        w(dCI, 16 * ((r - 1) * NQB + qb + 1))  # cco chunk free
                        if no_coll:
                            # timing probe only: local copy standing in for RS
                            gpsimd.dma_start(
                                cco[qb * 512 : (qb + 1) * 512, :],
                                cci[qb * 1024 : qb * 1024 + 512, :],
                            ).then_inc(sRS, RSU)
                        else:
                            gpsimd.collective_compute(
                                "ReduceScatter",
                                mybir.AluOpType.add,
                                replica_groups=RG,
                                ins=[cci[qb * 1024 : (qb + 1) * 1024, :]],
                                outs=[cco[qb * 512 : (qb + 1) * 512, :]],
                            ).then_inc(sRS, RSU)
                    for qb in range(1, NQB):
                        bias_add(r, qb)

            @blk.tensor
            def _(tensor):
                w = Waiter(tensor)
                for r in range(reps):
                    # ---- projections ----
                    def dxq_val(dt):
                        return 256 + 128 * r  # bulk: DMA completions unordered

                    for j, item in enumerate(proj):
                        gj = r * NPJ + j
                        is_v = item[0] == "v"
                        bank = (j % 4) if not is_v else 4 + (j % 4)
                        pslc = slice(bank * 512, bank * 512 + 512)
                        w(sPJC, gj - 3)
                        if j == 0:
                            w(sEX, NSTEPS * r)      # banks 0-3 free
                        if is_v and item[1] == 0:
                            w(sNM, 2 * NBLK * r)    # banks 4-5 free
                            w(sOD, 32 * r)          # banks 6-7 free
                        if not is_v:
                            kind, hp, sb = item
                            wt = WQt if kind == "q" else WKt
                            hsl = slice(hp * 128, (hp + 1) * 128)
                            ssl = slice(sb * 512, (sb + 1) * 512)
                            for dt in range(NDT):
                                w(dXQ, dxq_val(dt))
                                if kind == "k":
                                    w(dWK, 128)
                                mm = nc.tensor.matmul(
                                    P[:, pslc],
                                    lhsT=wt[dt][:, hsl],
                                    rhs=XT[dt][:, ssl],
                                    start=(dt == 0),
                                    stop=(dt == NDT - 1),
                                    skip_group_check=True,
                                )
                            mm.then_inc(sPJ, 1)
                        else:
                            _, st = item
                            stsl = slice(st * 128, (st + 1) * 128)
                            for dt in range(NDT):
                                w(dXQ, dxq_val(dt))
                                w(dWV, 128)
                                mm = nc.tensor.matmul(
                                    P[:, pslc],
                                    lhsT=XT[dt][:, stsl],
                                    rhs=WVt[dt][:],
                                    start=(dt == 0),
                                    stop=(dt == NDT - 1),
                                    skip_group_check=True,
                                )
                            mm.then_inc(sPJ, 1)

                    # ---- attention (qb-major) + interleaved out-proj ----
                    w(sPJC, (r + 1) * NPJ)
                    w(dMISC, 48)
                    w(dWO, 64)

                    def emit_scores(i):
                        qb, hp, kt, nkt, delta, qt_min = steps[i]
                        gi = r * NSTEPS + i
                        s = i % 2
                        qsl = slice(qb * 512, (qb + 1) * 512)
                        ksl = slice(kt * 128, (kt + 1) * 128)
                        w(sEX, gi - 1)
                        diag = delta >= 0
                        for rr in range(2):
                            psl = slice(rr * 64, (rr + 1) * 64)
                            mm = nc.tensor.matmul(
                                P[:, s * 1024 + rr * 512 : s * 1024 + rr * 512 + 512],
                                lhsT=KT[hp][psl, ksl],
                                rhs=QT[hp][psl, qsl],
                                start=True,
                                stop=not diag,
                                tile_position=(rr * 64, 0),
                                skip_group_check=True,
                            )
                        if diag:
                            for rr in range(2):
                                base = s * 1024 + rr * 512
                                mm = nc.tensor.matmul(
                                    P[:, base + delta : base + delta + 128],
                                    lhsT=NEGI[:],
                                    rhs=LTRI[:],
                                    start=False,
                                    stop=True,
                                    skip_group_check=True,
                                )
                        mm.then_inc(sPS, 1)

                    def emit_op_mm(sqb, g, dt):
                        # one out-projection matmul: group g = (qt, half)
                        qt, half = g // 2, g % 2
                        gidx = r * 32 + sqb * 8 + g
                        if dt == 0:
                            w(dAT, 64 * (r * NQB + sqb + 1))
                            w(sOD, gidx - 1)  # psum bank free (group gidx-2 drained)
                        mm = nc.tensor.matmul(
                            P[:, OPB + (g % 2) * 512 : OPB + (g % 2) * 512 + 512],
                            lhsT=ATB1[:, dt * 512 + qt * 128 : dt * 512 + qt * 128 + 128],
                            rhs=WO2[dt][:, half * 512 : (half + 1) * 512],
                            start=(dt == 0),
                            stop=(dt == 3),
                            skip_group_check=True,
                        )
                        if dt == 3:
                            mm.then_inc(sOP, 1)

                    emit_scores(0)
                    for i, (qb, hp, kt, nkt, delta, qt_min) in enumerate(steps):
                        gi = r * NSTEPS + i
                        b = qb * NHP + hp
                        gb = r * NBLK + b
                        if i + 1 < NSTEPS:
                            emit_scores(i + 1)
                        w(sEX, gi + 1)
                        # start=True clears has_written for the WHOLE bank, so
                        # only the first matmul into each bank may use it; the
                        # other regions open with start=False (cleared bits =>
                        # overwrite + set) and accumulate from the next kt on.
                        started_banks = set()
                        last_rr_qt = (1, 3)
                        for rr in range(2):
                            h = 2 * hp + rr
                            for qt in range(qt_min, 4):
                                bank = qt // 2
                                if kt == 0 and rr == 0 and qt == 0:
                                    # AV banks free: prev block drained
                                    w(sNM, 2 * gb)
                                st = kt == 0 and bank not in started_banks
                                if st:
                                    started_banks.add(bank)
                                mm = nc.tensor.matmul(
                                    P[0:128, avcol(qt) + rr * 65 : avcol(qt) + rr * 65 + 65],
                                    lhsT=PT[i % 6][:, rr * 512 + qt * 128 : rr * 512 + qt * 128 + 128],
                                    rhs=VS[kt][:, h * 65 : h * 65 + 65],
                                    start=st,
                                    stop=(kt == 4 * qb + qt),
                                    skip_group_check=True,
                                )
                                if (rr, qt) == last_rr_qt:
                                    mm.then_inc(sAV, 1)
                        # interleave the previous qb's out-proj work for this
                        # head-pair's groups (2hp, 2hp+1) near the START of the
                        # section so the drains (and the chunk's ReduceScatter)
                        # fire as early as possible.  1 matmul per step (2 for
                        # the short qb1 sections), starting at kt==2.
                        if qb > 0 and kt >= 2:
                            per = 2 if nkt == 8 else 1
                            for u in range(per * (kt - 2), min(per * (kt - 1), 8)):
                                emit_op_mm(qb - 1, 2 * hp + u // 4, u % 4)
                    # tail: last qb's out-projection
                    for g in range(8):
                        for dt in range(4):
                            emit_op_mm(NQB - 1, g, dt)

            @blk.scalar
            def _(scalar):
                w = Waiter(scalar)

                def readback(r, qb):
                    # cco chunk -> CSB on the ACT hwdge queue (idle during tail)
                    gq = r * NQB + qb
                    w(sRS, RSU * (gq + 1))
                    w(sOB, gq - 1)  # CSB[qb%2] free (bias-add of gq-2 done)
                    cv = cco[qb * 512 : (qb + 1) * 512, :].rearrange(
                        "(a b) c -> b a c", a=4
                    )
                    sv = CSB[qb % 2][:, :].rearrange("p (a c) -> p a c", a=4)
                    scalar.dma_start(sv, cv).then_inc(dCI, 16)

                for r in range(reps):
                    for i, (qb, hp, kt, nkt, delta, qt_min) in enumerate(steps):
                        if i == NSTEPS // 2:
                            readback(r, 0)  # RS(qb0) long done by now
                        gi = r * NSTEPS + i
                        w0 = max(delta, 0)
                        s = i % 2
                        w(sPS, gi + 1)
                        w(sAV, gi - 5)
                        src = P[:, s * 1024 : (s + 1) * 1024]
                        dst = PT[i % 6][:, :]
                        if w0 == 0:
                            act = nc.scalar.activation(dst, src, AF.Exp, scale=SCALE)
                        else:
                            sv = src.rearrange("p (t c) -> p t c", t=2)[:, :, w0:512]
                            dv = dst.rearrange("p (t c) -> p t c", t=2)[:, :, w0:512]
                            act = nc.scalar.activation(dv, sv, AF.Exp, scale=SCALE)
                        act.then_inc(sEX, 1)
                    # tail: drain the last qb's out-proj psums (ACT is idle
                    # once the exps are done; DVE must stay clear for the next
                    # rep's proj copies) and fetch the remaining RS outputs.
                    for g in range(8):
                        gidx = r * 32 + (NQB - 1) * 8 + g
                        w(sOP, gidx + 1)
                        w(dOC, 16 * (gidx - 3))  # OSB[gidx%4] free
                        nc.scalar.activation(
                            OSB[gidx % 4][:],
                            P[:, OPB + (g % 2) * 512 : OPB + (g % 2) * 512 + 512],
                            AF.Copy,
                        ).then_inc(sOD, 1)
                    for qb in range(1, NQB):
                        readback(r, qb)

            @blk.vector
            def _(vector):
                w = Waiter(vector)
                for st in range(NST):
                    vv = VS[st][:, :].rearrange("p (h x) -> p h x", x=65)
                    nc.vector.memset(vv[:, :, 64:65], 1.0)

                def drain(r, sqb, g):
                    gidx = r * 32 + sqb * 8 + g
                    w(sOP, gidx + 1)
                    w(dOC, 16 * (gidx - 3))  # OSB[gidx%4] free
                    nc.vector.tensor_copy(
                        OSB[gidx % 4][:], P[:, OPB + (g % 2) * 512 : OPB + (g % 2) * 512 + 512]
                    ).then_inc(sOD, 1)

                for r in range(reps):
                    for j, item in enumerate(proj):
                        gj = r * NPJ + j
                        bank = (j % 4) if item[0] != "v" else 4 + (j % 4)
                        pslc = slice(bank * 512, bank * 512 + 512)
                        w(sPJ, gj + 1)
                        if item[0] in ("q", "k"):
                            kind, hp, sb = item
                            if r > 0:
                                w(sPS, (r - 1) * NSTEPS + hp_last[hp] + 1)
                            dst = (QT if kind == "q" else KT)[hp]
                            ssl = slice(sb * 512, (sb + 1) * 512)
                            nc.vector.tensor_copy(dst[:, ssl], P[:, pslc]).then_inc(
                                sPJC, 1
                            )
                        else:
                            _, st = item
                            if r > 0:
                                w(sAV, (r - 1) * NSTEPS + vs_last[st] + 1)
                            vv = VS[st][:, :].rearrange("p (h x) -> p h x", x=65)
                            nc.vector.tensor_copy(
                                vv[:, :, 0:64],
                                P[:, pslc].rearrange("p (h x) -> p h x", x=64),
                            ).then_inc(sPJC, 1)
                    for qb in range(NQB):
                        for hp in range(NHP):
                            b = qb * NHP + hp
                            gb = r * NBLK + b
                            boff = 8 * (gb % 2)
                            w(dAT, 64 * (r * NQB + qb))  # ASB free (prev qb transposed)
                            if dbg is not None and r == 0 and qb == 1:
                                w(dMISC, 64)  # dbg2 snapshot read before overwrite

                            def recip(p):
                                sumc = P[:, avcol(2 * p) + 64 : avcol(2 * p) + 65]
                                nc.vector.reciprocal(
                                    RCM[:, boff + 4 * p : boff + 4 * p + 4].rearrange(
                                        "p (a b c) -> p a b c", a=2, b=2
                                    ),
                                    apv(sumc, [[130, 2], [65, 2], [1, 1]]),
                                )

                            def nmul(p):
                                src = P[:, avcol(2 * p) : avcol(2 * p) + 1]
                                rbase = RCM[:, boff + 4 * p : boff + 4 * p + 1]
                                abase = ASB[:, 1024 * p + hp * 128 : 1024 * p + hp * 128 + 1]
                                nc.vector.tensor_mul(
                                    apv(abase, [[512, 2], [64, 2], [1, 64]]),
                                    apv(src, [[130, 2], [65, 2], [1, 64]]),
                                    apv(rbase, [[2, 2], [1, 2], [0, 64]]),
                                ).then_inc(sNM, 1)

                            # NOTE: a DVE op must NOT read SBUF written by the
                            # immediately preceding DVE op (pipeline RAW) — keep
                            # at least one instruction between recip(p) and
                            # nmul(p).  Pair p is final at kt = 4qb + 2p + 1.
                            w(sAV, r * NSTEPS + block_last[b])
                            recip(0)
                            if qb > 0:
                                drain(r, qb - 1, 2 * hp)
                                nmul(0)
                                w(sAV, r * NSTEPS + block_last[b] + 1)
                                if dbg4 is not None and r == 0 and b == 0:
                                    nc.vector.tensor_copy(
                                        DBGP[:], P[:, 2048:3072]
                                    ).then_inc(sDB, 1)
                                recip(1)
                                drain(r, qb - 1, 2 * hp + 1)
                                nmul(1)
                            else:
                                w(sAV, r * NSTEPS + block_last[b] + 1)
                                if dbg4 is not None and r == 0 and b == 0:
                                    nc.vector.tensor_copy(
                                        DBGP[:], P[:, 2048:3072]
                                    ).then_inc(sDB, 1)
                                recip(1)
                                nmul(0)
                                nmul(1)

    return nc


_cached = {}


def _get_program(S=S_FULL, reps=1):
    key = (S, reps)
    if key not in _cached:
        _cached[key] = build_program(S, reps)
    return _cached[key]


def make_in_maps(x, Wq, Wk, Wv, Wo, bo):
    bf = ml_dtypes.bfloat16
    ltri01 = np.tril(np.ones((128, 128)), -1).astype(bf)
    negi01 = (np.eye(128) * -60000.0).astype(bf)
    x = np.asarray(x)
    xbb = [np.asarray(x[b]).astype(bf) for b in range(B)]
    Wq, Wk, Wv, Wo = (np.asarray(a) for a in (Wq, Wk, Wv, Wo))
    bo = np.asarray(bo, np.float32)
    in_maps = []
    for c in range(NCORES):
        b, p = divmod(c, 2)
        dsl = slice(p * DOWN, (p + 1) * DOWN)
        in_maps.append(
            {
                "xb": xbb[b],
                "wq": Wq[:, dsl].astype(bf),
                "wk": Wk[:, dsl].astype(bf),
                "wv": Wv[:, dsl].astype(bf),
                "wo2": Wo[dsl, :].astype(bf),
                "bob2": np.tile(bo[dsl], (128, 4)).astype(np.float32),
                "ltri": ltri01,
                "negi": negi01,
            }
        )
    return in_maps


def assemble(results, S):
    out = np.empty((B, S, D), np.float32)
    for c in range(NCORES):
        b, p = divmod(c, 2)
        out[b, :, p * DOWN : (p + 1) * DOWN] = results[c]["out"]
    return out


def kernel(**inputs):
    x = np.asarray(inputs["x"], np.float32)
    S = x.shape[1]
    nc = _get_program(S)
    in_maps = make_in_maps(
        x,
        inputs["Wq"],
        inputs["Wk"],
        inputs["Wv"],
        inputs["Wo"],
        inputs["bo"],
    )
    res = run_bass_kernel_spmd(nc, in_maps, core_ids=list(range(NCORES)))
    return assemble(res.results, S)


# revision 59
# speedup vs baseline: 1.1016x; 1.1016x over previous
"""Trainium2 Bass kernel for nn_MultiHeadCausalAttention (B=4, S=2048, D=1024, H=16).

Sharding: 8 cores = 4 (batch) x 2 (tensor-parallel over heads; 8 heads/core).
Per core (vs. the earlier AllGather design):
  - x arrives untransposed; x^T tiles are built on-chip with the DMA
    transpose XBAR (no host-side transpose).
  - QKV projections for the core's 8 heads.
  - Flash-style causal attention, scores in S^T layout (k on partitions) so
    exp(S^T) tiles feed the AV matmuls as lhsT.  AV outputs A in [q, d]
    layout (queries on partitions) via per-128-query-tile matmuls whose rhs
    is V|ones, so softmax row sums ride along in an extra column.
  - Normalisation is a reciprocal + broadcast multiply on DVE writing
    normalised A (bf16) to SBUF; the DMA XBAR transposes A -> A^T tiles.
  - Partial out-projection over the LOCAL 512 head-dims for all 1024 output
    columns, then a ReduceScatter(add) per 512-query chunk combines the two
    TP halves — the collective is the last step of each chunk and overlaps
    the attention of later chunks (query-block-major schedule).
  - Out-projection matmuls are interleaved one-per-attention-step into the
    next query block so the PE never idles waiting for exp.
Raw Bass (no Tile): per-engine programs with hand-placed counting semaphores.
`reps` replays the body inside one NEFF (sem values offset per rep) so the
true per-iteration time can be measured as a slope, independent of the
axon dispatch floor.
"""

from contextlib import ExitStack

import numpy as np
import ml_dtypes

import concourse.bass as bass
import concourse.mybir as mybir
from concourse.bass_utils import run_bass_kernel_spmd

F32 = mybir.dt.float32
BF16 = mybir.dt.bfloat16
AF = mybir.ActivationFunctionType

B, S_FULL, D = 4, 2048, 1024
NCORES = 8
NDT = D // 128         # 8 x-tiles along the model dim
DOWN = D // 2          # head-dims owned per core (8 heads * 64)
SCALE = 1.0 / 32.0     # d_out ** -0.5
RG = [[0, 1], [2, 3], [4, 5], [6, 7]]
NHP = 4                # head pairs per core


class Waiter:
    """Per-engine wait helper that elides waits already implied."""

    def __init__(self, eng):
        self.eng = eng
        self.seen = {}

    def __call__(self, sem, val):
        if val <= 0:
            return
        if self.seen.get(sem.name, -1) >= val:
            return
        self.seen[sem.name] = val
        self.eng.wait_ge(sem, val)


def apv(base_ap, dims):
    """Manual AP: keep base's partition dim, replace free dims ([stride, count])."""
    return bass.AP(base_ap.tensor, base_ap.offset, [list(base_ap.ap[0])] + dims)


def build_program(S=S_FULL, reps=1, debug=False, no_coll=False):
    NQB = S // 512         # query blocks
    NST = S // 128         # seq tiles
    NBLK = NQB * NHP       # (qb, hp) blocks per rep

    # qb-major attention step list
    steps = []
    block_last = []        # last step index of each block
    for qb in range(NQB):
        nkt = (qb + 1) * 4
        for hp in range(NHP):
            for kt in range(nkt):
                delta = kt * 128 - qb * 512
                qt_min = max(0, (delta + 127) // 128)
                steps.append((qb, hp, kt, nkt, delta, qt_min))
            block_last.append(len(steps) - 1)
    NSTEPS = len(steps)

    # last attention step reading QT/KT of hp (scores) and VS[st] (AV)
    hp_last = [max(i for i, s in enumerate(steps) if s[1] == hp) for hp in range(NHP)]
    vs_last = [max(i for i, s in enumerate(steps) if s[2] == st) for st in range(NST)]

    # proj emission order shared by PE and DVE
    proj = []
    for hp in range(NHP):
        for sb in range(S // 512):
            proj.append(("q", hp, sb))
            proj.append(("k", hp, sb))
    for st in range(NST):
        proj.append(("v", st))
    NPJ = len(proj)

    # psum column map (128 x 4096 f32 = 8 banks of 512)
    AVB = 2048             # AV regions in banks 4-5

    def avcol(qt):
        return AVB + (qt // 2) * 512 + (qt % 2) * 130

    OPB = 3072             # out-proj regions ping-pong banks 6-7
    RSU = 16 if no_coll else 1

    nc = bass.Bass()
    xb = nc.declare_dram_parameter("xb", [S, D], BF16, isOutput=False)
    wq = nc.declare_dram_parameter("wq", [D, DOWN], BF16, isOutput=False)
    wk = nc.declare_dram_parameter("wk", [D, DOWN], BF16, isOutput=False)
    wv = nc.declare_dram_parameter("wv", [D, DOWN], BF16, isOutput=False)
    wo2 = nc.declare_dram_parameter("wo2", [DOWN, D], BF16, isOutput=False)
    bob2 = nc.declare_dram_parameter("bob2", [128, 4 * DOWN], F32, isOutput=False)
    ltri = nc.declare_dram_parameter("ltri", [128, 128], BF16, isOutput=False)
    negi = nc.declare_dram_parameter("negi", [128, 128], BF16, isOutput=False)
    out = nc.declare_dram_parameter("out", [S, DOWN], F32, isOutput=True)

    # cci rows: qb*1024 + half*512 + local_q — each qb's ReduceScatter input
    # is the contiguous [1024, 512] block (dest-rank halves adjacent)
    cci = nc.dram_tensor("cci_rs", [2 * S, DOWN], BF16)
    cco = nc.dram_tensor("cco_rs", [S, DOWN], BF16)
    dbg = (
        nc.declare_dram_parameter("dbg", [2 * S, DOWN], BF16, isOutput=True)
        if debug
        else None
    )
    dbg2 = (
        nc.declare_dram_parameter("dbg2", [128, 2048], BF16, isOutput=True)
        if debug
        else None
    )
    dbg3 = (
        nc.declare_dram_parameter("dbg3", [128, 2048], BF16, isOutput=True)
        if debug
        else None
    )
    dbg4 = (
        nc.declare_dram_parameter("dbg4", [128, 1024], F32, isOutput=True)
        if debug
        else None
    )

    with ExitStack() as ctx:
        e = ctx.enter_context
        ctx.enter_context(
            nc.allow_low_precision(reason="intentional bf16 flash attention")
        )

        sems = {}
        for n in (
            "dXQ", "dWK", "dWV", "dWO", "dMISC", "sPJ", "sPJC", "sPS", "sEX",
            "sAV", "sNM", "dAT", "sOP", "sOD", "dOC", "sRS", "dCI", "sOB", "dO",
            "sDB", "sNT",
        ):
            sems[n] = e(nc.semaphore(n))
        sDB = sems["sDB"]
        sNT = sems["sNT"]
        dXQ, dWK, dWV, dWO, dMISC = (sems[k] for k in ("dXQ", "dWK", "dWV", "dWO", "dMISC"))
        sPJ, sPJC, sPS, sEX, sAV = (sems[k] for k in ("sPJ", "sPJC", "sPS", "sEX", "sAV"))
        sNM, dAT, sOP, sOD, dOC = (sems[k] for k in ("sNM", "dAT", "sOP", "sOD", "dOC"))
        sRS, dCI, sOB, dO = (sems[k] for k in ("sRS", "dCI", "sOB", "dO"))

        P = e(nc.psum_tensor("P", [128, 4096], F32))

        XT = [e(nc.sbuf_tensor(f"XT{i}", [128, S], BF16)) for i in range(NDT)]
        WQt = [e(nc.sbuf_tensor(f"WQt{i}", [128, DOWN], BF16)) for i in range(NDT)]
        WKt = [e(nc.sbuf_tensor(f"WKt{i}", [128, DOWN], BF16)) for i in range(NDT)]
        WVt = [e(nc.sbuf_tensor(f"WVt{i}", [128, DOWN], BF16)) for i in range(NDT)]
        WO2 = [e(nc.sbuf_tensor(f"WO2_{i}", [128, D], BF16)) for i in range(4)]
        QT = [e(nc.sbuf_tensor(f"QT{i}", [128, S], BF16)) for i in range(NHP)]
        KT = [e(nc.sbuf_tensor(f"KT{i}", [128, S], BF16)) for i in range(NHP)]
        VS = [e(nc.sbuf_tensor(f"VS{i}", [128, 8 * 65], BF16)) for i in range(NST)]
        PT = [e(nc.sbuf_tensor(f"PT{i}", [128, 1024], BF16)) for i in range(6)]
        RCM = e(nc.sbuf_tensor("RCM", [128, 16], F32))
        SCR = e(nc.sbuf_tensor("SCR", [1, 8], F32))
        SCA = e(nc.sbuf_tensor("SCA", [1, 8], F32))
        ASB = e(nc.sbuf_tensor("ASB", [128, 2048], BF16))
        ATB1 = e(nc.sbuf_tensor("ATB1", [128, 2048], BF16))
        OSB = [e(nc.sbuf_tensor(f"OSB{i}", [128, 512], BF16)) for i in range(4)]
        CSB = [e(nc.sbuf_tensor(f"CSB{i}", [128, 2048], BF16)) for i in range(2)]
        OUB = [e(nc.sbuf_tensor(f"OUB{i}", [128, 2048], F32)) for i in range(2)]
        DBGP = e(nc.sbuf_tensor("DBGP", [128, 1024], F32)) if debug else None
        LTRI = e(nc.sbuf_tensor("LTRI", [128, 128], BF16))
        NEGI = e(nc.sbuf_tensor("NEGI", [128, 128], BF16))
        BOB = e(nc.sbuf_tensor("BOB", [128, 4 * DOWN], F32))

        with nc.Block() as blk:

            @blk.sync
            def _(sync):
                w = Waiter(sync)
                for r in range(reps):
                    if r == 0:
                        for i in range(NDT):
                            sync.dma_start_transpose(
                                XT[i][:], xb[:, i * 128 : (i + 1) * 128]
                            ).then_inc(dXQ, 16)
                        for i in range(NDT):
                            sl = slice(i * 128, (i + 1) * 128)
                            sync.dma_start(WQt[i][:], wq[sl, :]).then_inc(dXQ, 16)
                        for i in range(NDT):
                            sl = slice(i * 128, (i + 1) * 128)
                            sync.dma_start(WKt[i][:], wk[sl, :]).then_inc(dWK, 16)
                        for i in range(NDT):
                            sl = slice(i * 128, (i + 1) * 128)
                            sync.dma_start(WVt[i][:], wv[sl, :]).then_inc(dWV, 16)
                        for i in range(4):
                            sl = slice(i * 128, (i + 1) * 128)
                            sync.dma_start(WO2[i][:], wo2[sl, :]).then_inc(dWO, 16)
                        sync.dma_start(LTRI[:], ltri[:]).then_inc(dMISC, 16)
                        sync.dma_start(NEGI[:], negi[:]).then_inc(dMISC, 16)
                        sync.dma_start(BOB[:], bob2[:]).then_inc(dMISC, 16)
                    # A^T tile transposes per query block: ASB [q, (qt,dt,d)]
                    # -> ATB1 [d, (dt,qt,q)] via the DMA XBAR, one call per qt
                    for qb in range(NQB):
                        if dbg2 is not None and r == 0 and qb == 0:
                            w(sNT, r * NBLK + (qb + 1) * NHP)
                            sync.dma_start(dbg2[:], ASB[:]).then_inc(dMISC, 16)
                        if dbg3 is not None and r == 0 and qb == 1:
                            w(sNT, r * NBLK + (qb + 1) * NHP)
                            sync.dma_start(dbg3[:], ATB1[:]).then_inc(dMISC, 16)
                            w(dMISC, 80)  # dbg3 read before overwrite
                        for qt in range(4):
                            # sNT rises one DVE op after the last normalise
                            # mul: settles the SBUF write before the XBAR read
                            w(sNT, r * NBLK + (qb + 1) * NHP)
                            if qt == 0:
                                w(sOP, 8 * (r * NQB + qb))  # ATB1 free
                            sync.dma_start_transpose(
                                apv(ATB1[:, qt * 128 : qt * 128 + 1], [[512, 4], [1, 128]]),
                                ASB[:, qt * 512 : (qt + 1) * 512],
                            ).then_inc(dAT, 16)
                    if r + 1 < reps:
                        w(sPJ, (r + 1) * NPJ)  # this rep's proj done reading XT
                        for i in range(NDT):
                            sync.dma_start_transpose(
                                XT[i][:], xb[:, i * 128 : (i + 1) * 128]
                            ).then_inc(dXQ, 16)
                    for qb in range(NQB):
                        gq = r * NQB + qb
                        w(sOB, gq + 1)
                        ov = out[qb * 512 : (qb + 1) * 512, :].rearrange(
                            "(a b) c -> b a c", a=4
                        )
                        uv = OUB[qb % 2][:, :].rearrange("p (a c) -> p a c", a=4)
                        sync.dma_start(ov, uv).then_inc(dO, 16)
                w(dO, 16 * reps * NQB)
                if dbg is not None:
                    w(dOC, 16 * reps * 32)
                    sync.dma_start(dbg[:], cci[:]).then_inc(dMISC, 16)
                if dbg4 is not None:
                    w(sDB, 1)
                    sync.dma_start(dbg4[:], DBGP[:]).then_inc(dMISC, 16)

            @blk.gpsimd
            def _(gpsimd):
                w = Waiter(gpsimd)

                def bias_add(r, qb):
                    # cco+bias -> OUB on Pool (SBUF-only engine; keeps the
                    # rep-tail off DVE so the next rep's proj copies can run)
                    gq = r * NQB + qb
                    w(dCI, 16 * (gq + 1))
                    w(dO, 16 * (gq - 1))  # OUB[qb%2] free
                    gpsimd.tensor_add(
                        OUB[qb % 2][:], CSB[qb % 2][:], BOB[:]
                    ).then_inc(sOB, 1)

                for r in range(reps):
                    for qb in range(NQB):
                        if qb == NQB - 1 and r * NQB + qb >= 3:
                            bias_add(*divmod(r * NQB + qb - 3, NQB))
                        for g in range(8):
                            qt, half = g // 2, g % 2
                            gidx = r * 32 + qb * 8 + g
                            w(sOD, gidx + 1)
                            if g == 0:
                                w(sRS, RSU * ((r - 1) * NQB + qb + 1))  # cci chunk free
                            row = qb * 1024 + half * 512 + qt * 128
                            gpsimd.dma_start(
                                cci[row : row + 128, :],
                                OSB[gidx % 4][:],
                            ).then_inc(dOC, 16)
                        w(dOC, 16 * (r * 32 + qb * 8 + 8))
                        w(dCI, 16 * ((r - 1) * NQB + qb + 1))  # cco chunk free
                        if no_coll:
                            # timing probe only: local copy standing in for RS
                            gpsimd.dma_start(
                                cco[qb * 512 : (qb + 1) * 512, :],
                                cci[qb * 1024 : qb * 1024 + 512, :],
                            ).then_inc(sRS, RSU)
                        else:
                            gpsimd.collective_compute(
                                "ReduceScatter",
                                mybir.AluOpType.add,
                                replica_groups=RG,
                                ins=[cci[qb * 1024 : (qb + 1) * 1024, :]],
                                outs=[cco[qb * 512 : (qb + 1) * 512, :]],
                            ).then_inc(sRS, RSU)
                    for qb in range(1, NQB):
                        bias_add(r, qb)

            @blk.tensor
            def _(tensor):
                w = Waiter(tensor)
                for r in range(reps):
                    # ---- projections ----
                    def dxq_val(dt):
                        return 256 + 128 * r  # bulk: DMA completions unordered

                    for j, item in enumerate(proj):
                        gj = r * NPJ + j
                        is_v = item[0] == "v"
                        bank = (j % 4) if not is_v else 4 + (j % 4)
                        pslc = slice(bank * 512, bank * 512 + 512)
                        w(sPJC, gj - 3)
                        if j == 0:
                            w(sEX, NSTEPS * r)      # banks 0-3 free
                        if is_v and item[1] == 0:
                            w(sNM, 2 * NBLK * r)    # banks 4-5 free
                            w(sOD, 32 * r)          # banks 6-7 free
                        if not is_v:
                            kind, hp, sb = item
                            wt = WQt if kind == "q" else WKt
                            hsl = slice(hp * 128, (hp + 1) * 128)
                            ssl = slice(sb * 512, (sb + 1) * 512)
                            for dt in range(NDT):
                                w(dXQ, dxq_val(dt))
                                if kind == "k":
                                    w(dWK, 128)
                                mm = nc.tensor.matmul(
                                    P[:, pslc],
                                    lhsT=wt[dt][:, hsl],
                                    rhs=XT[dt][:, ssl],
                                    start=(dt == 0),
                                    stop=(dt == NDT - 1),
                                    skip_group_check=True,
                                )
                            mm.then_inc(sPJ, 1)
                        else:
                            _, st = item
                            stsl = slice(st * 128, (st + 1) * 128)
                            for dt in range(NDT):
                                w(dXQ, dxq_val(dt))
                                w(dWV, 128)
                                mm = nc.tensor.matmul(
                                    P[:, pslc],
                                    lhsT=XT[dt][:, stsl],
                                    rhs=WVt[dt][:],
                                    start=(dt == 0),
                                    stop=(dt == NDT - 1),
                                    skip_group_check=True,
                                )
                            mm.then_inc(sPJ, 1)

                    # ---- attention (qb-major) + interleaved out-proj ----
                    w(sPJC, (r + 1) * NPJ)
                    w(dMISC, 48)
                    w(dWO, 64)

                    def emit_scores(i):
                        qb, hp, kt, nkt, delta, qt_min = steps[i]
                        gi = r * NSTEPS + i
                        s = i % 2
                        qsl = slice(qb * 512, (qb + 1) * 512)
                        ksl = slice(kt * 128, (kt + 1) * 128)
                        w(sEX, gi - 1)
                        diag = delta >= 0
                        for rr in range(2):
                            psl = slice(rr * 64, (rr + 1) * 64)
                            mm = nc.tensor.matmul(
                                P[:, s * 1024 + rr * 512 : s * 1024 + rr * 512 + 512],
                                lhsT=KT[hp][psl, ksl],
                                rhs=QT[hp][psl, qsl],
                                start=True,
                                stop=not diag,
                                tile_position=(rr * 64, 0),
                                skip_group_check=True,
                            )
                        if diag:
                            for rr in range(2):
                                base = s * 1024 + rr * 512
                                mm = nc.tensor.matmul(
                                    P[:, base + delta : base + delta + 128],
                                    lhsT=NEGI[:],
                                    rhs=LTRI[:],
                                    start=False,
                                    stop=True,
                                    skip_group_check=True,
                                )
                        mm.then_inc(sPS, 1)

                    def emit_op_mm(sqb, g, dt):
                        # one out-projection matmul: group g = (qt, half)
                        qt, half = g // 2, g % 2
                        gidx = r * 32 + sqb * 8 + g
                        if dt == 0:
                            w(dAT, 64 * (r * NQB + sqb + 1))
                            w(sOD, gidx - 1)  # psum bank free (group gidx-2 drained)
                        mm = nc.tensor.matmul(
                            P[:, OPB + (g % 2) * 512 : OPB + (g % 2) * 512 + 512],
                            lhsT=ATB1[:, dt * 512 + qt * 128 : dt * 512 + qt * 128 + 128],
                            rhs=WO2[dt][:, half * 512 : (half + 1) * 512],
                            start=(dt == 0),
                            stop=(dt == 3),
                            skip_group_check=True,
                        )
                        if dt == 3:
                            mm.then_inc(sOP, 1)

                    emit_scores(0)
                    for i, (qb, hp, kt, nkt, delta, qt_min) in enumerate(steps):
                        gi = r * NSTEPS + i
                        b = qb * NHP + hp
                        gb = r * NBLK + b
                        if i + 1 < NSTEPS:
                            emit_scores(i + 1)
                        w(sEX, gi + 1)
                        # start=True clears has_written for the WHOLE bank, so
                        # only the first matmul into each bank may use it; the
                        # other regions open with start=False (cleared bits =>
                        # overwrite + set) and accumulate from the next kt on.
                        started_banks = set()
                        last_rr_qt = (1, 3)
                        for rr in range(2):
                            h = 2 * hp + rr
                            for qt in range(qt_min, 4):
                                bank = qt // 2
                                if kt == 0 and rr == 0 and qt == 0:
                                    # AV banks free: prev block drained
                                    w(sNM, 2 * gb)
                                st = kt == 0 and bank not in started_banks
                                if st:
                                    started_banks.add(bank)
                                mm = nc.tensor.matmul(
                                    P[0:128, avcol(qt) + rr * 65 : avcol(qt) + rr * 65 + 65],
                                    lhsT=PT[i % 6][:, rr * 512 + qt * 128 : rr * 512 + qt * 128 + 128],
                                    rhs=VS[kt][:, h * 65 : h * 65 + 65],
                                    start=st,
                                    stop=(kt == 4 * qb + qt),
                                    skip_group_check=True,
                                )
                                if (rr, qt) == last_rr_qt:
                                    mm.then_inc(sAV, 1)
                        # interleave the previous qb's out-proj work for this
                        # head-pair's groups (2hp, 2hp+1) near the START of the
                        # section so the drains (and the chunk's ReduceScatter)
                        # fire as early as possible.  1 matmul per step (2 for
                        # the short qb1 sections), starting at kt==2.
                        if qb > 0 and kt >= 2:
                            per = 2 if nkt == 8 else 1
                            for u in range(per * (kt - 2), min(per * (kt - 1), 8)):
                                emit_op_mm(qb - 1, 2 * hp + u // 4, u % 4)
                    # tail: last qb's out-projection
                    for g in range(8):
                        for dt in range(4):
                            emit_op_mm(NQB - 1, g, dt)

            @blk.scalar
            def _(scalar):
                w = Waiter(scalar)

                def readback(r, qb):
                    # cco chunk -> CSB on the ACT hwdge queue (idle during tail)
                    gq = r * NQB + qb
                    w(sRS, RSU * (gq + 1))
                    w(sOB, gq - 1)  # CSB[qb%2] free (bias-add of gq-2 done)
                    cv = cco[qb * 512 : (qb + 1) * 512, :].rearrange(
                        "(a b) c -> b a c", a=4
                    )
                    sv = CSB[qb % 2][:, :].rearrange("p (a c) -> p a c", a=4)
                    scalar.dma_start(sv, cv).then_inc(dCI, 16)

                for r in range(reps):
                    for i, (qb, hp, kt, nkt, delta, qt_min) in enumerate(steps):
                        if i == NSTEPS // 2:
                            readback(r, 0)  # RS(qb0) long done by now
                        gi = r * NSTEPS + i
                        w0 = max(delta, 0)
                        s = i % 2
                        w(sPS, gi + 1)
                        w(sAV, gi - 5)
                        src = P[:, s * 1024 : (s + 1) * 1024]
                        dst = PT[i % 6][:, :]
                        if w0 == 0:
                            act = nc.scalar.activation(dst, src, AF.Exp, scale=SCALE)
                        else:
                            sv = src.rearrange("p (t c) -> p t c", t=2)[:, :, w0:512]
                            dv = dst.rearrange("p (t c) -> p t c", t=2)[:, :, w0:512]
                            act = nc.scalar.activation(dv, sv, AF.Exp, scale=SCALE)
                        act.then_inc(sEX, 1)
                    # tail: fetch the remaining RS outputs (the last qb's
                    # psum drains live at the head of the next DVE rep body)
                    for qb in range(1, NQB):
                        readback(r, qb)

            @blk.vector
            def _(vector):
                w = Waiter(vector)
                for st in range(NST):
                    vv = VS[st][:, :].rearrange("p (h x) -> p h x", x=65)
                    nc.vector.memset(vv[:, :, 64:65], 1.0)

                def drain(r, sqb, g):
                    gidx = r * 32 + sqb * 8 + g
                    w(sOP, gidx + 1)
                    w(dOC, 16 * (gidx - 3))  # OSB[gidx%4] free
                    nc.vector.memset(SCR[:], 0.0)  # psum settle after the wait
                    nc.vector.tensor_copy(
                        OSB[gidx % 4][:], P[:, OPB + (g % 2) * 512 : OPB + (g % 2) * 512 + 512]
                    ).then_inc(sOD, 1)

                for r in range(reps):
                    if r > 0:
                        for g in range(8):
                            drain(r - 1, NQB - 1, g)
                    for j, item in enumerate(proj):
                        gj = r * NPJ + j
                        bank = (j % 4) if item[0] != "v" else 4 + (j % 4)
                        pslc = slice(bank * 512, bank * 512 + 512)
                        w(sPJ, gj + 1)
                        if item[0] in ("q", "k"):
                            kind, hp, sb = item
                            if r > 0:
                                w(sPS, (r - 1) * NSTEPS + hp_last[hp] + 1)
                            dst = (QT if kind == "q" else KT)[hp]
                            ssl = slice(sb * 512, (sb + 1) * 512)
                            nc.vector.tensor_copy(dst[:, ssl], P[:, pslc]).then_inc(
                                sPJC, 1
                            )
                        else:
                            _, st = item
                            if r > 0:
                                w(sAV, (r - 1) * NSTEPS + vs_last[st] + 1)
                            vv = VS[st][:, :].rearrange("p (h x) -> p h x", x=65)
                            nc.vector.tensor_copy(
                                vv[:, :, 0:64],
                                P[:, pslc].rearrange("p (h x) -> p h x", x=64),
                            ).then_inc(sPJC, 1)
                    for qb in range(NQB):
                        for hp in range(NHP):
                            b = qb * NHP + hp
                            gb = r * NBLK + b
                            boff = 8 * (gb % 2)
                            w(dAT, 64 * (r * NQB + qb))  # ASB free (prev qb transposed)
                            if dbg is not None and r == 0 and qb == 1:
                                w(dMISC, 64)  # dbg2 snapshot read before overwrite

                            def recip(p):
                                sumc = P[:, avcol(2 * p) + 64 : avcol(2 * p) + 65]
                                nc.vector.reciprocal(
                                    RCM[:, boff + 4 * p : boff + 4 * p + 4].rearrange(
                                        "p (a b c) -> p a b c", a=2, b=2
                                    ),
                                    apv(sumc, [[130, 2], [65, 2], [1, 1]]),
                                )

                            def nmul(p):
                                src = P[:, avcol(2 * p) : avcol(2 * p) + 1]
                                rbase = RCM[:, boff + 4 * p : boff + 4 * p + 1]
                                abase = ASB[:, 1024 * p + hp * 128 : 1024 * p + hp * 128 + 1]
                                nc.vector.tensor_mul(
                                    apv(abase, [[512, 2], [64, 2], [1, 64]]),
                                    apv(src, [[130, 2], [65, 2], [1, 64]]),
                                    apv(rbase, [[2, 2], [1, 2], [0, 64]]),
                                ).then_inc(sNM, 1)

                            # NOTE: a DVE op must NOT read SBUF written by the
                            # immediately preceding DVE op (pipeline RAW) — keep
                            # at least one instruction between recip(p) and
                            # nmul(p).  Pair p is final at kt = 4qb + 2p + 1.
                            w(sAV, r * NSTEPS + block_last[b])
                            recip(0)
                            if qb > 0:
                                drain(r, qb - 1, 2 * hp)
                                nmul(0)
                                w(sAV, r * NSTEPS + block_last[b] + 1)
                                nc.vector.memset(SCR[:], 0.0)  # psum settle
                                if dbg4 is not None and r == 0 and b == 0:
                                    nc.vector.tensor_copy(
                                        DBGP[:], P[:, 2048:3072]
                                    ).then_inc(sDB, 1)
                                recip(1)
                                drain(r, qb - 1, 2 * hp + 1)
                                nmul(1)
                                nc.vector.memset(SCR[:], 0.0).then_inc(sNT, 1)
                            else:
                                w(sAV, r * NSTEPS + block_last[b] + 1)
                                nc.vector.memset(SCR[:], 0.0)  # psum settle
                                if dbg4 is not None and r == 0 and b == 0:
                                    nc.vector.tensor_copy(
                                        DBGP[:], P[:, 2048:3072]
                                    ).then_inc(sDB, 1)
                                recip(1)
                                nmul(0)
                                nmul(1)
                                nc.vector.memset(SCR[:], 0.0).then_inc(sNT, 1)
                for g in range(8):
                    drain(reps - 1, NQB - 1, g)

    return nc


_cached = {}


def _get_program(S=S_FULL, reps=1):
    key = (S, reps)
    if key not in _cached:
        _cached[key] = build_program(S, reps)
    return _cached[key]


def make_in_maps(x, Wq, Wk, Wv, Wo, bo):
    bf = ml_dtypes.bfloat16
    ltri01 = np.tril(np.ones((128, 128)), -1).astype(bf)
    negi01 = (np.eye(128) * -60000.0).astype(bf)
    x = np.asarray(x)
    xbb = [np.asarray(x[b]).astype(bf) for b in range(B)]
    Wq, Wk, Wv, Wo = (np.asarray(a) for a in (Wq, Wk, Wv, Wo))
    bo = np.asarray(bo, np.float32)
    in_maps = []
    for c in range(NCORES):
        b, p = divmod(c, 2)
        dsl = slice(p * DOWN, (p + 1) * DOWN)
        in_maps.append(
            {
                "xb": xbb[b],
                "wq": Wq[:, dsl].astype(bf),
                "wk": Wk[:, dsl].astype(bf),
                "wv": Wv[:, dsl].astype(bf),
                "wo2": Wo[dsl, :].astype(bf),
                "bob2": np.tile(bo[dsl], (128, 4)).astype(np.float32),
                "ltri": ltri01,
                "negi": negi01,
            }
        )
    return in_maps


def assemble(results, S):
    out = np.empty((B, S, D), np.float32)
    for c in range(NCORES):
        b, p = divmod(c, 2)
        out[b, :, p * DOWN : (p + 1) * DOWN] = results[c]["out"]
    return out


def kernel(**inputs):
    x = np.asarray(inputs["x"], np.float32)
    S = x.shape[1]
    nc = _get_program(S)
    in_maps = make_in_maps(
        x,
        inputs["Wq"],
        inputs["Wk"],
        inputs["Wv"],
        inputs["Wo"],
        inputs["bo"],
    )
    res = run_bass_kernel_spmd(nc, in_maps, core_ids=list(range(NCORES)))
    return assemble(res.results, S)
